# revision 10
# baseline (speedup 1.0000x reference)
"""ConditionalFilterLayer Bass/Tile kernel for 8 Trainium2 NeuronCores.

Strategy: pure data parallel over batch (1 sample per core). All four GEMM
stages are expressed in "transposed" form so the PE output free size is the
tiny class dim (k=19) instead of the spatial dim, and DMA traffic is minimized
(x is loaded once in bf16 c-major + once in fp8 s-major at half scale).

Per core, with X = x[i] viewed as [C=512, S=16384]:
  1. pre_maskT[s,k] = X^T Wm^T + bm   (lhsT = resident bf16 X blocks [c,s],
     rhs = WmT [c,19] moving, bias via a rank-1 ones x bm matmul; sigmoid on
     ACT engine straight into an fp8 maskT [s,k] tile)
  2. cfT[c,k] = sum_s (x/2)[s,c] * maskT[s,k]  (lhsT = streamed fp8 xT blocks,
     rhs = fp8 maskT [s,19]; 4 PSUM accumulation chains over 128 s-blocks)
     cf is re-quantized to fp8 as hi + lo halves (error-compensated split) at
     scale 1/4 so values stay in fp8e4m3 normal range.
  3. filters[k,o] = Wf[k] cf[k] + bf  (lhsT = fp8 Wf blocks [c,o], rhs = fp8
     cfT columns n=1; the pooling 1/S and fp8 scales fold into a 1/4096
     activation-copy scale at PSUM evacuation)
  4. predT[s,k] = X^T filters^T  (lhsT = resident X blocks, rhs = bf16
     filtersT [c,19] moving; bf16 output, host does the final transpose)

Measured numerics vs the fp32 reference: rel err ~1.45e-2 (gate 2e-2).
"""

import contextlib

import numpy as np
import ml_dtypes

import concourse.bass as bass
import concourse.tile as tile
from concourse import mybir
from concourse.bass_utils import run_bass_kernel_spmd
from concourse.vector_clock import ScopedClock

B, C, K, H, W = 8, 512, 19, 128, 128
S = H * W                    # 16384
NCT = C // 128               # 4 c-chunks
NJ = S // 128                # 128 s-blocks
SH = S // 2                  # xc DMA half
NXT = 4                      # xT8 stream tiles
XTJ = NJ // NXT              # 32 s-blocks per xT8 tile
# s-block grouping for step 1 / step 4 PSUM tiles
JT = [(0, 26), (26, 26), (52, 26), (78, 26), (104, 24)]
# class grouping for Wf stream tiles
KG = [(0, 4), (4, 4), (8, 4), (12, 4), (16, 3)]
N_CORES = 8

F32 = mybir.dt.float32
BF16 = mybir.dt.bfloat16
F8 = mybir.dt.float8e4

npbf16 = ml_dtypes.bfloat16
npf8 = ml_dtypes.float8_e4m3


class TC(tile.TileContext):
    """TileContext whose exit drain carries at most one sync wait per
    instruction — this walrus build rejects multi-wait CTRL ops."""

    def _drain_and_barrier(self, tick_clock, wait_clock):
        nc = self.nc
        drain_inst = nc.sync.drain()
        wait_clock.add_sem_waits(
            drain_inst.ins, ScopedClock({None: tick_clock.global_clock})
        )
        si = drain_inst.ins.sync_info
        waits = list(si.on_wait) if si else []
        if len(waits) > 1:
            SyncInfo = type(si)
            drain_inst.ins.sync_info = SyncInfo(on_wait=[waits[0]], on_update=[])
            for w in waits[1:]:
                n = nc.sync.nop(nofuse=True, hint="split_drain_wait")
                n.ins.sync_info = SyncInfo(on_wait=[w], on_update=[])
        nc.all_engine_barrier()
        assert self.sems is not None
        popped = nc._tile_sem_poison_stack.pop()
        assert popped is self._sem_poison
        nc.clear_and_free_semaphores(list(self.sems.allocated().values()))
        nc.all_engine_barrier()


def _split_multiwaits(nc, max_waits=1):
    """This walrus build rejects instructions with more than one sync wait:
    peel extra waits onto same-engine no-ops inserted just before."""
    import bass_rust
    for f in nc.m.functions:
        for bb in f.blocks:
            insts = list(bb.instructions)
            out, changed = [], False
            for inst in insts:
                si = inst.sync_info
                waits = list(si.on_wait) if si else []
                if len(waits) > max_waits:
                    for w in waits[:-max_waits]:
                        n = mybir.InstNoOp(
                            name=f"I-wsplit-{nc.next_id()}", ins=[], outs=[]
                        )
                        n.engine = inst.engine
                        n.sync_info = bass_rust.SyncInfo(on_wait=[w], on_update=[])
                        out.append(n)
                    inst.sync_info = bass_rust.SyncInfo(
                        on_wait=waits[-max_waits:], on_update=list(si.on_update)
                    )
                    changed = True
                out.append(inst)
            if changed:
                bb.instructions = out


PHASE_MARKS = {}


def _build_kernel():
    nc = bass.Bass("TRN2", target_bir_lowering=False, debug=False)
    PHASE_MARKS.clear()

    def mark(name):
        PHASE_MARKS[name] = nc.next_id()

    xc_d = nc.dram_tensor("xc", [C, S], BF16, kind="ExternalInput").ap()
    xT8_d = nc.dram_tensor("xT8", [128, NJ, C], F8, kind="ExternalInput").ap()
    wm_d = nc.dram_tensor("wmT", [128, NCT, K], BF16, kind="ExternalInput").ap()
    ob_d = nc.dram_tensor("onesbm", [1, 128 + K], BF16, kind="ExternalInput").ap()
    wf8_d = nc.dram_tensor("wf8", [K, 128, NCT, C], F8, kind="ExternalInput").ap()
    bf_d = nc.dram_tensor("bfT", [128, NCT, K], F32, kind="ExternalInput").ap()
    pred_d = nc.dram_tensor("pred", [128, NJ, K], BF16, kind="ExternalOutput").ap()

    with TC(nc) as tc, contextlib.ExitStack() as ctx:
        const_p = ctx.enter_context(tc.tile_pool(name="const", bufs=1))
        xc_p = ctx.enter_context(tc.tile_pool(name="xc", bufs=1))
        xt_p = ctx.enter_context(tc.tile_pool(name="xt", bufs=2))
        wf_p = ctx.enter_context(tc.tile_pool(name="wf", bufs=2))
        mask_p = ctx.enter_context(tc.tile_pool(name="mask", bufs=1))
        cf_p = ctx.enter_context(tc.tile_pool(name="cf", bufs=1))
        out_p = ctx.enter_context(tc.tile_pool(name="outp", bufs=1))
        pm_p = ctx.enter_context(tc.tile_pool(name="ps_pm", bufs=2, space="PSUM"))
        cfps_p = ctx.enter_context(tc.tile_pool(name="ps_cf", bufs=1, space="PSUM"))
        fps_p = ctx.enter_context(tc.tile_pool(name="ps_f", bufs=1, space="PSUM"))
        pp_p = ctx.enter_context(tc.tile_pool(name="ps_p", bufs=2, space="PSUM"))

        # --- small consts (Activation-engine DMAs, finish early) ---
        wm_sb = const_p.tile([128, NCT, K], BF16, tag="wm")
        nc.scalar.dma_start(wm_sb[:], wm_d[:])
        ob_sb = const_p.tile([1, 128 + K], BF16, tag="ob")
        nc.scalar.dma_start(ob_sb[:], ob_d[:])
        bf_sb = const_p.tile([128, NCT, K], F32, tag="bf")
        nc.scalar.dma_start(bf_sb[:], bf_d[:])
        ones = ob_sb[:, 0:128]
        bmr = ob_sb[:, 128:128 + K]

        # --- resident bf16 x (c-major), 8 DMAs so step 1 starts at half ---
        xc = {}
        for h in range(2):
            for ct in range(NCT):
                t = xc_p.tile([128, SH], BF16, tag=f"xc{ct}_{h}", name=f"xc{ct}_{h}")
                nc.sync.dma_start(
                    t[:], xc_d[ct * 128:(ct + 1) * 128, h * SH:(h + 1) * SH]
                )
                xc[(ct, h)] = t

        def xblk(ct, j):
            h, off = divmod(j, NJ // 2)
            return xc[(ct, h)][:, off * 128:(off + 1) * 128]

        # --- streamed fp8 xT (s-major, x/2) -- issued after xc on SP queue ---
        xt_tiles = []
        for t in range(NXT):
            xt = xt_p.tile([128, XTJ, C], F8)
            nc.sync.dma_start(xt[:], xT8_d[:, t * XTJ:(t + 1) * XTJ, :])
            xt_tiles.append(xt)

        # --- streamed fp8 Wf -- issued after xT8 on SP queue ---
        wf_tiles = []
        for (k0, nk) in KG:
            wf = wf_p.tile([128, nk, NCT, C], F8)
            nc.sync.dma_start(
                wf[:], wf8_d[k0:k0 + nk].rearrange("k p t o -> p k t o")
            )
            wf_tiles.append(wf)

        mark("A_start")
        # --- step 1: pre_maskT blocks + sigmoid -> fp8 maskT hi+lo [s, k] ---
        m_hi = mask_p.tile([128, NJ, K], F8, tag="mhi")
        m_lo = mask_p.tile([128, NJ, K], F8, tag="mlo")
        msc_p = ctx.enter_context(tc.tile_pool(name="msc", bufs=2))
        for (j0, nb) in JT:
            pm = pm_p.tile([128, nb, K], F32)
            for jj in range(nb):
                j = j0 + jj
                for ct in range(NCT):
                    nc.tensor.matmul(
                        pm[:, jj, :], lhsT=xblk(ct, j), rhs=wm_sb[:, ct, :],
                        start=(ct == 0), stop=False,
                    )
                nc.tensor.matmul(
                    pm[:, jj, :], lhsT=ones, rhs=bmr, start=False, stop=True,
                )
            sl = (slice(None), slice(j0, j0 + nb), slice(None))
            mbf = msc_p.tile([128, nb, K], BF16, tag="mbf")
            mhif = msc_p.tile([128, nb, K], F32, tag="mhif")
            mres = msc_p.tile([128, nb, K], F32, tag="mres")
            nc.scalar.activation(
                mbf[:], pm[:], mybir.ActivationFunctionType.Sigmoid,
            )
            nc.scalar.activation(
                m_hi[sl], mbf[:], mybir.ActivationFunctionType.Copy,
            )
            nc.vector.tensor_copy(mhif[:], m_hi[sl])
            nc.vector.tensor_sub(mres[:], mbf[:], mhif[:])
            nc.vector.tensor_copy(m_lo[sl], mres[:])

        mark("B_start")
        # --- step 2: cfT[c,k] accumulation over 128 s-blocks, hi+lo mask ---
        cfp = cfps_p.tile([128, NCT, K], F32)
        for t in range(NXT):
            xt = xt_tiles[t]
            for jl in range(XTJ):
                j = t * XTJ + jl
                for ct in range(NCT):
                    for mi, mq in enumerate((m_hi, m_lo)):
                        nc.tensor.matmul(
                            cfp[:, ct, :],
                            lhsT=xt[:, jl, ct * 128:(ct + 1) * 128],
                            rhs=mq[:, j, :],
                            start=(j == 0 and mi == 0),
                            stop=(j == NJ - 1 and mi == 1),
                        )
        # psum = 8*S*cf_true; quantize 2*S*cf to fp8 hi + lo halves
        cf_hi = cf_p.tile([128, NCT, K], F8, tag="cfhi")
        cf_tmp = cf_p.tile([128, NCT, K], F32, tag="cftmp")
        cf_hif = cf_p.tile([128, NCT, K], F32, tag="cfhif")
        cf_res = cf_p.tile([128, NCT, K], F32, tag="cfres")
        cf_lo = cf_p.tile([128, NCT, K], F8, tag="cflo")
        nc.scalar.activation(
            cf_hi[:], cfp[:], mybir.ActivationFunctionType.Copy, scale=1.0 / 32.0
        )
        nc.scalar.activation(
            cf_tmp[:], cfp[:], mybir.ActivationFunctionType.Copy, scale=1.0 / 32.0
        )
        nc.vector.tensor_copy(cf_hif[:], cf_hi[:])
        nc.vector.tensor_sub(cf_res[:], cf_tmp[:], cf_hif[:])
        nc.vector.tensor_copy(cf_lo[:], cf_res[:])

        mark("C_start")
        # --- step 3: per-class filter GEMM, n=1 matmuls, hi+lo compensated ---
        fps = fps_p.tile([128, NCT, K], F32)
        for g, (k0, nk) in enumerate(KG):
            wf = wf_tiles[g]
            for kk in range(nk):
                k = k0 + kk
                for oc in range(NCT):
                    step = 0
                    for ct in range(NCT):
                        for cfq in (cf_hi, cf_lo):
                            nc.tensor.matmul(
                                fps[:, oc, k:k + 1],
                                lhsT=wf[:, kk, ct, oc * 128:(oc + 1) * 128],
                                rhs=cfq[:, ct, k:k + 1],
                                start=(step == 0), stop=(step == 2 * NCT - 1),
                            )
                            step += 1
        # filters = psum/(16*S) + bf  (covers the 8x xT, 1/32 cf, 64x Wf scales)
        ftmp = cf_p.tile([128, NCT, K], F32, tag="ftmp")
        filtT = cf_p.tile([128, NCT, K], BF16, tag="filtT")
        nc.scalar.activation(
            ftmp[:], fps[:], mybir.ActivationFunctionType.Copy,
            scale=1.0 / 262144.0,
        )
        nc.vector.tensor_add(filtT[:], ftmp[:], bf_sb[:])

        mark("D_start")
        # --- step 4: predT[s,k] blocks -> bf16 staging -> chunked DMA out ---
        stag = out_p.tile([128, NJ, K], BF16, tag="stag")
        for gi, (j0, nb) in enumerate(JT):
            pp = pp_p.tile([128, nb, K], F32)
            for jj in range(nb):
                j = j0 + jj
                for ct in range(NCT):
                    nc.tensor.matmul(
                        pp[:, jj, :], lhsT=xblk(ct, j), rhs=filtT[:, ct, :],
                        start=(ct == 0), stop=(ct == NCT - 1),
                    )
            nc.scalar.activation(
                stag[:, j0:j0 + nb, :], pp[:],
                mybir.ActivationFunctionType.Copy,
            )
            if gi >= 1:  # chunked output overlaps the step-4 tail
                c0 = 0 if gi == 1 else j0  # fold group 0 into group 1's chunk
                nc.sync.dma_start(
                    pred_d[:, c0:j0 + nb, :], stag[:, c0:j0 + nb, :]
                )

    mark("end")
    _split_multiwaits(nc)
    return nc


_NC_CACHE = None


def _prep_in_maps(x, Wm, bm, Wf, bf):
    # wmT[p, ct, k] = Wm[k, ct*128+p]
    wmT = np.ascontiguousarray(
        Wm.T.reshape(NCT, 128, K).transpose(1, 0, 2)
    ).astype(npbf16)
    onesbm = np.zeros((1, 128 + K), np.float32)
    onesbm[0, :128] = 1.0
    onesbm[0, 128:] = bm
    onesbm = onesbm.astype(npbf16)
    # wf8[k, c_local, ct, o] = 64*Wf[k, o, ct*128+c_local]  (fp8, scaled away
    # from the e4m3 subnormal range; folded back at PSUM evacuation)
    wf8 = np.ascontiguousarray(
        (Wf * 64.0).transpose(0, 2, 1).reshape(K, NCT, 128, C).transpose(0, 2, 1, 3)
    ).astype(npf8)
    # bfT[o_local, oc, k] = bf[k, oc*128+o_local]
    bfT = np.ascontiguousarray(
        bf.T.reshape(NCT, 128, K).transpose(1, 0, 2)
    ).astype(np.float32)
    maps = []
    for i in range(N_CORES):
        xi = x[i].reshape(C, S)
        xc = np.ascontiguousarray(xi).astype(npbf16)
        # xT8[p, j, c] = 8 * x[c, j*128+p]  (scaled out of fp8 subnormals)
        xT8 = np.ascontiguousarray(
            (xi.T * 8.0).reshape(NJ, 128, C).transpose(1, 0, 2)
        ).astype(npf8)
        maps.append({
            "xc": xc,
            "xT8": xT8,
            "wmT": wmT,
            "onesbm": onesbm,
            "wf8": wf8,
            "bfT": bfT,
        })
    return maps


def kernel(x, Wm, bm, Wf, bf):
    global _NC_CACHE
    if _NC_CACHE is None:
        _NC_CACHE = _build_kernel()
    nc = _NC_CACHE

    x = np.asarray(x, dtype=np.float32)
    in_maps = _prep_in_maps(
        x, np.asarray(Wm, np.float32), np.asarray(bm, np.float32),
        np.asarray(Wf, np.float32), np.asarray(bf, np.float32))

    res = run_bass_kernel_spmd(nc, in_maps, list(range(N_CORES)))
    out = np.empty((N_CORES, K, H, W), np.float32)
    for i in range(N_CORES):
        arr = np.asarray(res.results[i]["pred"], dtype=np.float32)  # [p, j, k]
        out[i] = arr.transpose(2, 1, 0).reshape(K, H, W)
    return out


def time_kernel(inputs, iters=20):
    """Steady-state per-launch wall time (ns) with device-resident inputs."""
    import jax
    from jax.sharding import Mesh, PartitionSpec, NamedSharding
    from jax.experimental.shard_map import shard_map
    from concourse import mybir as _mybir
    from concourse.bass2jax import (
        _bass_exec_p, install_neuronx_cc_hook, partition_id_tensor,
    )
    import time as _time

    global _NC_CACHE
    if _NC_CACHE is None:
        _NC_CACHE = _build_kernel()
    nc = _NC_CACHE
    install_neuronx_cc_hook()

    in_maps = _prep_in_maps(
        np.asarray(inputs["x"], np.float32), np.asarray(inputs["Wm"], np.float32),
        np.asarray(inputs["bm"], np.float32), np.asarray(inputs["Wf"], np.float32),
        np.asarray(inputs["bf"], np.float32))

    in_names, out_names, out_avals, zero_outs = [], [], [], []
    pid_name = nc.partition_id_tensor.name if nc.partition_id_tensor else None
    for alloc in nc.m.functions[0].allocations:
        if not isinstance(alloc, _mybir.MemoryLocationSet):
            continue
        name = alloc.memorylocations[0].name
        if alloc.kind == "ExternalInput":
            if name != pid_name:
                in_names.append(name)
        elif alloc.kind == "ExternalOutput":
            shape = tuple(alloc.tensor_shape)
            dt = _mybir.dt.np(alloc.dtype)
            out_names.append(name)
            out_avals.append(jax.core.ShapedArray(shape, dt))
            zero_outs.append(np.zeros(shape, dt))
    n_params = len(in_names)
    all_in_names = in_names + out_names
    if nc.partition_id_tensor is not None:
        all_in_names = all_in_names + [nc.partition_id_tensor.name]

    def _body(*args):
        operands = list(args)
        if nc.partition_id_tensor is not None:
            operands.append(partition_id_tensor())
        outs = _bass_exec_p.bind(
            *operands,
            out_avals=tuple(out_avals),
            in_names=tuple(all_in_names),
            out_names=tuple(out_names),
            lowering_input_output_aliases=(),
            sim_require_finite=True,
            sim_require_nnan=True,
            nc=nc,
        )
        return tuple(outs)

    devices = jax.devices()[:N_CORES]
    mesh = Mesh(np.asarray(devices), ("core",))
    spec = PartitionSpec("core")
    n_outs = len(out_names)
    sharded = jax.jit(
        shard_map(
            _body, mesh=mesh, in_specs=(spec,) * (n_params + n_outs),
            out_specs=(spec,) * n_outs, check_rep=False,
        ),
        keep_unused=True,
    )
    concat_in = [
        np.concatenate([np.asarray(in_maps[c][nm]) for c in range(N_CORES)], axis=0)
        for nm in in_names
    ]
    concat_zeros = [
        np.zeros((N_CORES * z.shape[0], *z.shape[1:]), z.dtype) for z in zero_outs
    ]
    sh = NamedSharding(mesh, spec)
    dev_in = [jax.device_put(a, sh) for a in concat_in + concat_zeros]
    out = sharded(*dev_in)
    jax.block_until_ready(out)
    t0 = _time.perf_counter()
    for _ in range(iters):
        out = sharded(*dev_in)
    jax.block_until_ready(out)
    dt = (_time.perf_counter() - t0) / iters
    return dt * 1e9
